# revision 17
# baseline (speedup 1.0000x reference)
"""Trainium2 Bass kernel for nn_BranchingLayer (gnn_message_passing).

Computation (reference):
    parents_ftxs = x[idxs_level]                      # identity gather (arange)
    pg           = global_features[parents_idxs % B]  # random gather
    h1 = leaky_relu([parents_ftxs, pg] @ W1 + b1)
    h2 = h1 @ W2 + b2 + repeat(parents_ftxs, 2, -1)
    children = interleave-reshape(h2)                 # [(2p+br)*B + b, f] = h2[p*B+b, br*F+f]
    out = concat([x, children])

Device strategy (8 cores, rows sharded 32768/core = 32 parents), v2:
  All compute stays in the transposed (feature-major) domain so every matmul
  has weights stationary and rows moving (N=512), all in bf16:
    - host: xT [128, rows], pgT duplicated into both partition halves
      [128, rows] (bf16), per core.
    - mm1: h1T[m] = W1x_m^T xT (+) W1g_m^T pgT, the two K=64 global-feature
      matmuls packed into one concurrent row-tile pair (tile_position 0/64).
    - ACT: leaky-relu + b1 (per-partition bias in this domain) -> h1 bf16.
    - mm2: h2T[oh] = W2 blocks^T h1 (+) residual via a 0/1 repeat matrix,
      packed as a second K=64 row-tile pair streaming xT halves.
    - DVE: + b2 (per-partition here) while casting PSUM -> bf16 SBUF.
    - out DMA: chT [2, 128, rows] bf16, one DMA per 4-group supergroup.
  Host untangles chT into child-row order and concats [x, children].
"""

import sys

import numpy as np

try:
    import ml_dtypes
except ImportError:
    ml_dtypes = None

if "/opt/trn_rl_repo" not in sys.path:
    sys.path.insert(0, "/opt/trn_rl_repo")

N_PARENTS = 256
BATCH = 1024
N_FEAT = 128
N_BR = 2
N_GLOBAL = 64
N_CORES = 8
ROWS = N_PARENTS * BATCH            # 262144
RPC = ROWS // N_CORES               # 32768 rows per core
PPC = N_PARENTS // N_CORES          # 32 parents per core
CPC = RPC * N_BR                    # 65536 child rows per core
GROUP = 512                         # rows per pipeline group
SGROUP = 4                          # groups per DMA supergroup
N_SG = RPC // (GROUP * SGROUP)      # 16
HID = 256

_CACHE = {}


def _split_multiwait(nc, mybir):
    """This image's walrus accepts only one sync-wait per instruction; hoist
    extra waits onto same-engine NOPs inserted before the instruction."""
    for f in nc.m.functions:
        for bb in f.blocks:
            new_insts = []
            changed = False
            for inst in bb.instructions:
                si = inst.sync_info
                if si is not None and len(si.on_wait) > 1:
                    waits = list(si.on_wait)
                    for w in waits[:-1]:
                        new_insts.append(
                            mybir.InstNoOp(
                                name=nc.get_next_instruction_name(),
                                engine=inst.engine,
                                sync_info=mybir.SyncInfo(on_wait=[w], on_update=[]),
                            )
                        )
                    inst.sync_info = mybir.SyncInfo(
                        on_wait=[waits[-1]], on_update=list(si.on_update)
                    )
                    changed = True
                new_insts.append(inst)
            if changed:
                bb.instructions = new_insts


def _build_program():
    key = ("prog_v2",)
    if key in _CACHE:
        return _CACHE[key]

    import concourse.bass as bass
    import concourse.mybir as mybir
    import concourse.tile as tile

    f32 = mybir.dt.float32
    bf16 = mybir.dt.bfloat16

    nc = bass.Bass()
    xt = nc.declare_dram_parameter("xt", [N_FEAT, RPC], bf16, isOutput=False)
    pgd = nc.declare_dram_parameter("pgd", [128, RPC], bf16, isOutput=False)
    wkb = nc.declare_dram_parameter("wkb", [128, 1024], bf16, isOutput=False)
    bkf = nc.declare_dram_parameter("bkf", [128, 4], f32, isOutput=False)
    cht = nc.declare_dram_parameter("cht", [2, 128, RPC], bf16, isOutput=True)

    AF = mybir.ActivationFunctionType

    with tile.TileContext(nc) as tc:
        with (
            tc.tile_pool(name="const", bufs=1) as cpool,
            tc.tile_pool(name="xin", bufs=2) as xpool,
            tc.tile_pool(name="gin", bufs=2) as gpool,
            tc.tile_pool(name="h1", bufs=6) as h1pool,
            tc.tile_pool(name="cout", bufs=2) as opool,
            tc.tile_pool(name="ps1", bufs=2, space="PSUM") as ps1,
            tc.tile_pool(name="ps2", bufs=2, space="PSUM") as ps2,
        ):
            wks = cpool.tile([128, 1024], bf16)
            nc.sync.dma_start(wks[:], wkb[:, :])
            bks = cpool.tile([128, 4], f32)
            nc.sync.dma_start(bks[:], bkf[:, :])
            w1xs = wks[:, 0:256]
            w2a = wks[:, 256:512]
            w2b = wks[:, 512:768]
            w1gs = wks[:, 768:896]
            rcs = wks[:, 896:1024]
            b1s = bks[:, 0:2]
            b2s = bks[:, 2:4]

            # PE warmup: ~3.5us of dummy matmuls on a never-written scratch
            # tile (values irrelevant, output discarded) so the HAM clock
            # gate reaches 8/8 before the first data tiles land. No DMA
            # dependency: starts right after the framework prologue.
            junk = cpool.tile([128, 2 * HID], bf16)
            nc.gpsimd.memset(junk[:, :], 0.0)
            warm = ps1.tile([128, 2 * GROUP], f32, tag="h1ps", name="warm")
            for _ in range(24):
                nc.tensor.matmul(
                    warm[:, 0:HID], junk[:, 0:128], junk[:, 256:512],
                    start=True, stop=True, skip_group_check=True,
                )

            # Software-pipelined emission (1-group skew): emit mm1(g)+lrelu(g)
            # BEFORE mm2(g-1). Scheduler priorities follow emission order and
            # the PE queue executes in order, so this keeps ready mm1 work in
            # front of mm2 instructions that wait on the lrelu chain.
            NG = N_SG * SGROUP
            sg_state = {}       # sg -> [og tile, og3 view, n_dve_done]
            prev = None
            prev2 = None

            def emit_mm2(st):
                xs_, h1, h2ps, og3, cs = (
                    st["xs"], st["h1"], st["h2ps"], st["og3"], st["cs"]
                )
                # residual K=64 pair first (depends only on xg)
                nc.tensor.matmul(
                    h2ps[0], rcs[0:64, :], xs_[0:64, :],
                    start=True, stop=False, tile_position=(0, 0),
                )
                nc.tensor.matmul(
                    h2ps[1], rcs[64:128, :], xs_[64:128, :],
                    start=True, stop=False, tile_position=(64, 0),
                )
                for oh in range(2):
                    nc.tensor.matmul(
                        h2ps[oh], w2a[:, oh * 128:(oh + 1) * 128],
                        h1[0][:, :], start=False, stop=False,
                    )
                for oh in range(2):
                    nc.tensor.matmul(
                        h2ps[oh], w2b[:, oh * 128:(oh + 1) * 128],
                        h1[1][:, :], start=False, stop=True,
                    )
                # + b2 (per-partition here) while casting to bf16;
                # oh0 on DVE, oh1 on ACT to balance the two engines
                nc.vector.tensor_scalar_add(
                    og3[:, 0, cs], h2ps[0], b2s[:, 0:1],
                )
                nc.scalar.activation(
                    og3[:, 1, cs], h2ps[1], AF.Identity,
                    bias=b2s[:, 1:2], scale=1.0,
                )
                # close out the supergroup's output DMA after its last group
                sgp = st["sg"]
                sg_state[sgp][2] += 1
                ndone = sg_state[sgp][2]
                if sgp == N_SG - 1:
                    hw = GROUP
                    cp0 = sgp * SGROUP * GROUP + (ndone - 1) * hw
                    cp1 = cp0 + hw
                    nc.sync.dma_start(
                        cht[:, :, cp0:cp1].rearrange("oh p c -> p oh c"),
                        sg_state[sgp][1][:, :, (ndone - 1) * hw:ndone * hw],
                    )
                    if ndone == SGROUP:
                        del sg_state[sgp]
                elif ndone in (SGROUP // 2, SGROUP):
                    half = 0 if ndone == SGROUP // 2 else 1
                    hw = SGROUP * GROUP // 2
                    cp0 = sgp * SGROUP * GROUP + half * hw
                    cp1 = cp0 + hw
                    nc.sync.dma_start(
                        cht[:, :, cp0:cp1].rearrange("oh p c -> p oh c"),
                        sg_state[sgp][1][:, :, half * hw:(half + 1) * hw],
                    )
                    if ndone == SGROUP:
                        del sg_state[sgp]

            for g in range(NG):
                sg, gl = divmod(g, SGROUP)
                if gl == 0:
                    c0 = sg * SGROUP * GROUP
                    c1 = c0 + SGROUP * GROUP
                    gg = gpool.tile([128, SGROUP * GROUP], bf16, tag="gg")
                    nc.gpsimd.dma_start(gg[:, :], pgd[:, c0:c1])
                    xg = xpool.tile([128, SGROUP * GROUP], bf16, tag="xg")
                    nc.gpsimd.dma_start(xg[:, :], xt[:, c0:c1])
                    og = opool.tile([128, 2 * SGROUP * GROUP], bf16, tag="og")
                    og3 = og[:, :].rearrange("p (oh c) -> p oh c", oh=2)
                    sg_state[sg] = [og, og3, 0]

                cs = slice(gl * GROUP, (gl + 1) * GROUP)
                xs_ = xg[:, cs]
                gs_ = gg[:, cs]

                # ---- mm1: h1T[m] = W1x_m^T @ xT + W1g_m^T @ pgT ----
                # The K=64 pair starts each accumulation group: it only
                # depends on the input DMA, so the scheduler can issue the
                # two halves adjacently (concurrent row-tiles).
                h1ps2 = ps1.tile([128, 2 * GROUP], f32, tag="h1ps")
                h1ps = [h1ps2[:, m_ * GROUP:(m_ + 1) * GROUP]
                        for m_ in range(2)]
                nc.tensor.matmul(
                    h1ps[0], w1gs[0:64, :], gs_[0:64, :],
                    start=True, stop=False, tile_position=(0, 0),
                )
                nc.tensor.matmul(
                    h1ps[1], w1gs[64:128, :], gs_[64:128, :],
                    start=True, stop=False, tile_position=(64, 0),
                )
                for m in range(2):
                    nc.tensor.matmul(
                        h1ps[m], w1xs[:, m * 128:(m + 1) * 128], xs_,
                        start=False, stop=True,
                    )

                # ---- leaky-relu + b1 (per-partition bias), cast bf16 ----
                h1 = [h1pool.tile([128, GROUP], bf16, tag="h1sb",
                                  name=f"h1sb{m_}") for m_ in range(2)]
                for m in range(2):
                    nc.scalar.activation(
                        h1[m][:, :], h1ps[m], AF.Lrelu,
                        bias=b1s[:, m:m + 1], scale=1.0, alpha=0.01,
                    )

                # mm2 tiles for this group (allocated now, filled next iter)
                h2ps2 = ps2.tile([128, 2 * GROUP], f32, tag="h2ps")
                h2ps = [h2ps2[:, o_ * GROUP:(o_ + 1) * GROUP]
                        for o_ in range(2)]
                cur = {"sg": sg, "cs": cs, "xs": xs_, "h1": h1,
                       "h2ps": h2ps, "og3": og3}

                if prev2 is not None:
                    emit_mm2(prev2)
                prev2 = prev
                prev = cur

            emit_mm2(prev2)
            emit_mm2(prev)

    import concourse.mybir as mybir
    _split_multiwait(nc, mybir)
    _CACHE[key] = nc
    return nc


def _host_prep(x, global_features, W1, b1, W2, b2, idxs_level, parents_idxs):
    bf = ml_dtypes.bfloat16
    x = np.ascontiguousarray(np.asarray(x, dtype=np.float32))
    G = np.asarray(global_features, dtype=np.float32)
    W1 = np.asarray(W1, dtype=np.float32)
    b1 = np.asarray(b1, dtype=np.float32)
    W2 = np.asarray(W2, dtype=np.float32)
    b2 = np.asarray(b2, dtype=np.float32)
    idxs = np.asarray(idxs_level)
    pidx = np.asarray(parents_idxs)

    if np.array_equal(idxs, np.arange(ROWS, dtype=idxs.dtype)):
        xg = x
    else:  # general gather fallback (host)
        xg = np.ascontiguousarray(x[idxs])

    # per-core transposed x: [8, 128, RPC]
    xt = np.ascontiguousarray(
        xg.reshape(N_CORES, RPC, N_FEAT).transpose(0, 2, 1)
    ).astype(bf)
    # per-core transposed gathered globals, duplicated into both halves
    pg = G[pidx % BATCH]                                  # [ROWS, 64]
    pgt = pg.reshape(N_CORES, RPC, N_GLOBAL).transpose(0, 2, 1).astype(bf)
    pgd = np.ascontiguousarray(np.concatenate([pgt, pgt], axis=1))  # [8,128,RPC]

    w1x = W1[0:128, :]                                     # [128, 256]
    w1g = np.concatenate([W1[128:192, 0:128], W1[128:192, 128:256]], axis=0)
    rp = np.zeros((64, 128), dtype=np.float32)
    rp[np.arange(128) // 2, np.arange(128)] = 1.0
    rcd = np.concatenate([rp, rp], axis=0)
    wkb = np.ascontiguousarray(np.concatenate(
        [w1x, W2[0:128, :], W2[128:256, :], w1g, rcd], axis=1
    )).astype(bf)                                          # [128, 1024]
    bkf = np.ascontiguousarray(np.concatenate(
        [b1.reshape(2, 128).T, b2.reshape(2, 128).T], axis=1
    ).astype(np.float32))                                  # [128, 4]

    in_maps = []
    for c in range(N_CORES):
        in_maps.append({
            "xt": xt[c],
            "pgd": pgd[c],
            "wkb": wkb,
            "bkf": bkf,
        })
    return x, in_maps


def kernel(x, global_features, W1, b1, W2, b2, idxs_level, parents_idxs,
           _trace=False, _trace_kwargs=None):
    from concourse.bass_utils import run_bass_kernel_spmd

    x_np, in_maps = _host_prep(
        x, global_features, W1, b1, W2, b2, idxs_level, parents_idxs
    )
    nc = _build_program()
    res = run_bass_kernel_spmd(
        nc, in_maps, list(range(N_CORES)),
        trace=_trace, **(_trace_kwargs or {}),
    )
    # cht[core][oh, f, pl*1024 + b] -> children[(core*32 + pl)*2 + oh, b, f]
    parts = []
    for c in range(N_CORES):
        a = np.asarray(res.results[c]["cht"])              # [2, 128, RPC] bf16
        a = a.reshape(2, 128, PPC, BATCH).transpose(2, 0, 3, 1)
        parts.append(a.reshape(CPC, N_FEAT).astype(np.float32))
    children = np.concatenate(parts, axis=0)
    out = np.concatenate([x_np, children], axis=0)
    if _trace:
        kernel.last_result = res
    return out


# revision 18
# speedup vs baseline: 1.0513x; 1.0513x over previous
"""Trainium2 Bass kernel for nn_BranchingLayer (gnn_message_passing).

Computation (reference):
    parents_ftxs = x[idxs_level]                      # identity gather (arange)
    pg           = global_features[parents_idxs % B]  # random gather
    h1 = leaky_relu([parents_ftxs, pg] @ W1 + b1)
    h2 = h1 @ W2 + b2 + repeat(parents_ftxs, 2, -1)
    children = interleave-reshape(h2)                 # [(2p+br)*B + b, f] = h2[p*B+b, br*F+f]
    out = concat([x, children])

Device strategy (8 cores, rows sharded 32768/core = 32 parents), v2:
  All compute stays in the transposed (feature-major) domain so every matmul
  has weights stationary and rows moving (N=512), all in bf16:
    - host: xT [128, rows], pgT duplicated into both partition halves
      [128, rows] (bf16), per core.
    - mm1: h1T[m] = W1x_m^T xT (+) W1g_m^T pgT, the two K=64 global-feature
      matmuls packed into one concurrent row-tile pair (tile_position 0/64).
    - ACT: leaky-relu + b1 (per-partition bias in this domain) -> h1 bf16.
    - mm2: h2T[oh] = W2 blocks^T h1 (+) residual via a 0/1 repeat matrix,
      packed as a second K=64 row-tile pair streaming xT halves.
    - DVE: + b2 (per-partition here) while casting PSUM -> bf16 SBUF.
    - out DMA: chT [2, 128, rows] bf16, one DMA per 4-group supergroup.
  Host untangles chT into child-row order and concats [x, children].
"""

import sys

import numpy as np

try:
    import ml_dtypes
except ImportError:
    ml_dtypes = None

if "/opt/trn_rl_repo" not in sys.path:
    sys.path.insert(0, "/opt/trn_rl_repo")

N_PARENTS = 256
BATCH = 1024
N_FEAT = 128
N_BR = 2
N_GLOBAL = 64
N_CORES = 8
ROWS = N_PARENTS * BATCH            # 262144
RPC = ROWS // N_CORES               # 32768 rows per core
PPC = N_PARENTS // N_CORES          # 32 parents per core
CPC = RPC * N_BR                    # 65536 child rows per core
GROUP = 512                         # rows per pipeline group
SGROUP = 4                          # groups per DMA supergroup
N_SG = RPC // (GROUP * SGROUP)      # 16
HID = 256

_CACHE = {}


def _split_multiwait(nc, mybir):
    """This image's walrus accepts only one sync-wait per instruction; hoist
    extra waits onto same-engine NOPs inserted before the instruction."""
    for f in nc.m.functions:
        for bb in f.blocks:
            new_insts = []
            changed = False
            for inst in bb.instructions:
                si = inst.sync_info
                if si is not None and len(si.on_wait) > 1:
                    waits = list(si.on_wait)
                    for w in waits[:-1]:
                        new_insts.append(
                            mybir.InstNoOp(
                                name=nc.get_next_instruction_name(),
                                engine=inst.engine,
                                sync_info=mybir.SyncInfo(on_wait=[w], on_update=[]),
                            )
                        )
                    inst.sync_info = mybir.SyncInfo(
                        on_wait=[waits[-1]], on_update=list(si.on_update)
                    )
                    changed = True
                new_insts.append(inst)
            if changed:
                bb.instructions = new_insts


def _build_program():
    key = ("prog_v2",)
    if key in _CACHE:
        return _CACHE[key]

    import concourse.bass as bass
    import concourse.mybir as mybir
    import concourse.tile as tile

    f32 = mybir.dt.float32
    bf16 = mybir.dt.bfloat16

    nc = bass.Bass()
    xt = nc.declare_dram_parameter("xt", [N_FEAT, RPC], bf16, isOutput=False)
    pgd = nc.declare_dram_parameter("pgd", [128, RPC], bf16, isOutput=False)
    wkb = nc.declare_dram_parameter("wkb", [128, 1024], bf16, isOutput=False)
    bkf = nc.declare_dram_parameter("bkf", [128, 4], f32, isOutput=False)
    cht = nc.declare_dram_parameter("cht", [2, 128, RPC], bf16, isOutput=True)

    AF = mybir.ActivationFunctionType

    with tile.TileContext(nc) as tc:
        with (
            tc.tile_pool(name="const", bufs=1) as cpool,
            tc.tile_pool(name="xin", bufs=2) as xpool,
            tc.tile_pool(name="gin", bufs=2) as gpool,
            tc.tile_pool(name="h1", bufs=6) as h1pool,
            tc.tile_pool(name="cout", bufs=2) as opool,
            tc.tile_pool(name="ps1", bufs=2, space="PSUM") as ps1,
            tc.tile_pool(name="ps2", bufs=2, space="PSUM") as ps2,
        ):
            wks = cpool.tile([128, 1024], bf16)
            nc.sync.dma_start(wks[:], wkb[:, :])
            bks = cpool.tile([128, 4], f32)
            nc.sync.dma_start(bks[:], bkf[:, :])
            w1xs = wks[:, 0:256]
            w2a = wks[:, 256:512]
            w2b = wks[:, 512:768]
            w1gs = wks[:, 768:896]
            rcs = wks[:, 896:1024]
            b1s = bks[:, 0:2]
            b2s = bks[:, 2:4]

            # PE warmup: ~3.5us of dummy matmuls on a never-written scratch
            # tile (values irrelevant, output discarded) so the HAM clock
            # gate reaches 8/8 before the first data tiles land. No DMA
            # dependency: starts right after the framework prologue.
            junk = cpool.tile([128, 2 * HID], bf16)
            nc.gpsimd.memset(junk[:, :], 0.0)
            warm = ps1.tile([128, 2 * GROUP], f32, tag="h1ps", name="warm")
            for _ in range(24):
                nc.tensor.matmul(
                    warm[:, 0:HID], junk[:, 0:128], junk[:, 256:512],
                    start=True, stop=True, skip_group_check=True,
                )

            # Software-pipelined emission (1-group skew): emit mm1(g)+lrelu(g)
            # BEFORE mm2(g-1). Scheduler priorities follow emission order and
            # the PE queue executes in order, so this keeps ready mm1 work in
            # front of mm2 instructions that wait on the lrelu chain.
            NG = N_SG * SGROUP
            sg_state = {}       # sg -> [og tile, og3 view, n_dve_done]
            prev = None
            prev2 = None

            def emit_mm2(st):
                xs_, h1, h2ps, og3, cs = (
                    st["xs"], st["h1"], st["h2ps"], st["og3"], st["cs"]
                )
                # residual K=64 pair first (depends only on xg)
                nc.tensor.matmul(
                    h2ps[0], rcs[0:64, :], xs_[0:64, :],
                    start=True, stop=False, tile_position=(0, 0),
                )
                nc.tensor.matmul(
                    h2ps[1], rcs[64:128, :], xs_[64:128, :],
                    start=True, stop=False, tile_position=(64, 0),
                )
                for oh in range(2):
                    nc.tensor.matmul(
                        h2ps[oh], w2a[:, oh * 128:(oh + 1) * 128],
                        h1[0][:, :], start=False, stop=False,
                    )
                for oh in range(2):
                    nc.tensor.matmul(
                        h2ps[oh], w2b[:, oh * 128:(oh + 1) * 128],
                        h1[1][:, :], start=False, stop=True,
                    )
                # + b2 (per-partition here) while casting to bf16
                for oh in range(2):
                    nc.vector.tensor_scalar_add(
                        og3[:, oh, cs], h2ps[oh], b2s[:, oh:oh + 1],
                    )
                # close out the supergroup's output DMA after its last group
                sgp = st["sg"]
                sg_state[sgp][2] += 1
                ndone = sg_state[sgp][2]
                if sgp == N_SG - 1:
                    hw = GROUP
                    cp0 = sgp * SGROUP * GROUP + (ndone - 1) * hw
                    cp1 = cp0 + hw
                    nc.sync.dma_start(
                        cht[:, :, cp0:cp1].rearrange("oh p c -> p oh c"),
                        sg_state[sgp][1][:, :, (ndone - 1) * hw:ndone * hw],
                    )
                    if ndone == SGROUP:
                        del sg_state[sgp]
                elif ndone in (SGROUP // 2, SGROUP):
                    half = 0 if ndone == SGROUP // 2 else 1
                    hw = SGROUP * GROUP // 2
                    cp0 = sgp * SGROUP * GROUP + half * hw
                    cp1 = cp0 + hw
                    nc.sync.dma_start(
                        cht[:, :, cp0:cp1].rearrange("oh p c -> p oh c"),
                        sg_state[sgp][1][:, :, half * hw:(half + 1) * hw],
                    )
                    if ndone == SGROUP:
                        del sg_state[sgp]

            for g in range(NG):
                sg, gl = divmod(g, SGROUP)
                if gl == 0:
                    c0 = sg * SGROUP * GROUP
                    c1 = c0 + SGROUP * GROUP
                    gg = gpool.tile([128, SGROUP * GROUP], bf16, tag="gg")
                    nc.gpsimd.dma_start(gg[:, :], pgd[:, c0:c1])
                    xg = xpool.tile([128, SGROUP * GROUP], bf16, tag="xg")
                    nc.gpsimd.dma_start(xg[:, :], xt[:, c0:c1])
                    og = opool.tile([128, 2 * SGROUP * GROUP], bf16, tag="og")
                    og3 = og[:, :].rearrange("p (oh c) -> p oh c", oh=2)
                    sg_state[sg] = [og, og3, 0]

                cs = slice(gl * GROUP, (gl + 1) * GROUP)
                xs_ = xg[:, cs]
                gs_ = gg[:, cs]

                # ---- mm1: h1T[m] = W1x_m^T @ xT + W1g_m^T @ pgT ----
                # The K=64 pair starts each accumulation group: it only
                # depends on the input DMA, so the scheduler can issue the
                # two halves adjacently (concurrent row-tiles).
                h1ps2 = ps1.tile([128, 2 * GROUP], f32, tag="h1ps")
                h1ps = [h1ps2[:, m_ * GROUP:(m_ + 1) * GROUP]
                        for m_ in range(2)]
                nc.tensor.matmul(
                    h1ps[0], w1gs[0:64, :], gs_[0:64, :],
                    start=True, stop=False, tile_position=(0, 0),
                )
                nc.tensor.matmul(
                    h1ps[1], w1gs[64:128, :], gs_[64:128, :],
                    start=True, stop=False, tile_position=(64, 0),
                )
                for m in range(2):
                    nc.tensor.matmul(
                        h1ps[m], w1xs[:, m * 128:(m + 1) * 128], xs_,
                        start=False, stop=True,
                    )

                # ---- leaky-relu + b1 (per-partition bias), cast bf16 ----
                h1 = [h1pool.tile([128, GROUP], bf16, tag="h1sb",
                                  name=f"h1sb{m_}") for m_ in range(2)]
                for m in range(2):
                    nc.scalar.activation(
                        h1[m][:, :], h1ps[m], AF.Lrelu,
                        bias=b1s[:, m:m + 1], scale=1.0, alpha=0.01,
                    )

                # mm2 tiles for this group (allocated now, filled next iter)
                h2ps2 = ps2.tile([128, 2 * GROUP], f32, tag="h2ps")
                h2ps = [h2ps2[:, o_ * GROUP:(o_ + 1) * GROUP]
                        for o_ in range(2)]
                cur = {"sg": sg, "cs": cs, "xs": xs_, "h1": h1,
                       "h2ps": h2ps, "og3": og3}

                if prev2 is not None:
                    emit_mm2(prev2)
                prev2 = prev
                prev = cur

            emit_mm2(prev2)
            emit_mm2(prev)

    import concourse.mybir as mybir
    _split_multiwait(nc, mybir)
    _CACHE[key] = nc
    return nc


def _host_prep(x, global_features, W1, b1, W2, b2, idxs_level, parents_idxs):
    bf = ml_dtypes.bfloat16
    x = np.ascontiguousarray(np.asarray(x, dtype=np.float32))
    G = np.asarray(global_features, dtype=np.float32)
    W1 = np.asarray(W1, dtype=np.float32)
    b1 = np.asarray(b1, dtype=np.float32)
    W2 = np.asarray(W2, dtype=np.float32)
    b2 = np.asarray(b2, dtype=np.float32)
    idxs = np.asarray(idxs_level)
    pidx = np.asarray(parents_idxs)

    if np.array_equal(idxs, np.arange(ROWS, dtype=idxs.dtype)):
        xg = x
    else:  # general gather fallback (host)
        xg = np.ascontiguousarray(x[idxs])

    # per-core transposed x: [8, 128, RPC]
    xt = np.ascontiguousarray(
        xg.reshape(N_CORES, RPC, N_FEAT).transpose(0, 2, 1)
    ).astype(bf)
    # per-core transposed gathered globals, duplicated into both halves
    pg = G[pidx % BATCH]                                  # [ROWS, 64]
    pgt = pg.reshape(N_CORES, RPC, N_GLOBAL).transpose(0, 2, 1).astype(bf)
    pgd = np.ascontiguousarray(np.concatenate([pgt, pgt], axis=1))  # [8,128,RPC]

    w1x = W1[0:128, :]                                     # [128, 256]
    w1g = np.concatenate([W1[128:192, 0:128], W1[128:192, 128:256]], axis=0)
    rp = np.zeros((64, 128), dtype=np.float32)
    rp[np.arange(128) // 2, np.arange(128)] = 1.0
    rcd = np.concatenate([rp, rp], axis=0)
    wkb = np.ascontiguousarray(np.concatenate(
        [w1x, W2[0:128, :], W2[128:256, :], w1g, rcd], axis=1
    )).astype(bf)                                          # [128, 1024]
    bkf = np.ascontiguousarray(np.concatenate(
        [b1.reshape(2, 128).T, b2.reshape(2, 128).T], axis=1
    ).astype(np.float32))                                  # [128, 4]

    in_maps = []
    for c in range(N_CORES):
        in_maps.append({
            "xt": xt[c],
            "pgd": pgd[c],
            "wkb": wkb,
            "bkf": bkf,
        })
    return x, in_maps


def kernel(x, global_features, W1, b1, W2, b2, idxs_level, parents_idxs,
           _trace=False, _trace_kwargs=None):
    from concourse.bass_utils import run_bass_kernel_spmd

    x_np, in_maps = _host_prep(
        x, global_features, W1, b1, W2, b2, idxs_level, parents_idxs
    )
    nc = _build_program()
    res = run_bass_kernel_spmd(
        nc, in_maps, list(range(N_CORES)),
        trace=_trace, **(_trace_kwargs or {}),
    )
    # cht[core][oh, f, pl*1024 + b] -> children[(core*32 + pl)*2 + oh, b, f]
    parts = []
    for c in range(N_CORES):
        a = np.asarray(res.results[c]["cht"])              # [2, 128, RPC] bf16
        a = a.reshape(2, 128, PPC, BATCH).transpose(2, 0, 3, 1)
        parts.append(a.reshape(CPC, N_FEAT).astype(np.float32))
    children = np.concatenate(parts, axis=0)
    out = np.concatenate([x_np, children], axis=0)
    if _trace:
        kernel.last_result = res
    return out


# revision 19
# speedup vs baseline: 1.1364x; 1.0810x over previous
"""Trainium2 Bass kernel for nn_BranchingLayer (gnn_message_passing).

Computation (reference):
    parents_ftxs = x[idxs_level]                      # identity gather (arange)
    pg           = global_features[parents_idxs % B]  # random gather
    h1 = leaky_relu([parents_ftxs, pg] @ W1 + b1)
    h2 = h1 @ W2 + b2 + repeat(parents_ftxs, 2, -1)
    children = interleave-reshape(h2)                 # [(2p+br)*B + b, f] = h2[p*B+b, br*F+f]
    out = concat([x, children])

Device strategy (8 cores, rows sharded 32768/core = 32 parents), v2:
  All compute stays in the transposed (feature-major) domain so every matmul
  has weights stationary and rows moving (N=512), all in bf16:
    - host: xT [128, rows], pgT duplicated into both partition halves
      [128, rows] (bf16), per core.
    - mm1: h1T[m] = W1x_m^T xT (+) W1g_m^T pgT, the two K=64 global-feature
      matmuls packed into one concurrent row-tile pair (tile_position 0/64).
    - ACT: leaky-relu + b1 (per-partition bias in this domain) -> h1 bf16.
    - mm2: h2T[oh] = W2 blocks^T h1 (+) residual via a 0/1 repeat matrix,
      packed as a second K=64 row-tile pair streaming xT halves.
    - DVE: + b2 (per-partition here) while casting PSUM -> bf16 SBUF.
    - out DMA: chT [2, 128, rows] bf16, one DMA per 4-group supergroup.
  Host untangles chT into child-row order and concats [x, children].
"""

import sys

import numpy as np

try:
    import ml_dtypes
except ImportError:
    ml_dtypes = None

if "/opt/trn_rl_repo" not in sys.path:
    sys.path.insert(0, "/opt/trn_rl_repo")

N_PARENTS = 256
BATCH = 1024
N_FEAT = 128
N_BR = 2
N_GLOBAL = 64
N_CORES = 8
ROWS = N_PARENTS * BATCH            # 262144
RPC = ROWS // N_CORES               # 32768 rows per core
PPC = N_PARENTS // N_CORES          # 32 parents per core
CPC = RPC * N_BR                    # 65536 child rows per core
GROUP = 512                         # rows per pipeline group
SGROUP = 4                          # groups per DMA supergroup
N_SG = RPC // (GROUP * SGROUP)      # 16
HID = 256

_CACHE = {}


def _split_multiwait(nc, mybir):
    """This image's walrus accepts only one sync-wait per instruction; hoist
    extra waits onto same-engine NOPs inserted before the instruction."""
    for f in nc.m.functions:
        for bb in f.blocks:
            new_insts = []
            changed = False
            for inst in bb.instructions:
                si = inst.sync_info
                if si is not None and len(si.on_wait) > 1:
                    waits = list(si.on_wait)
                    for w in waits[:-1]:
                        new_insts.append(
                            mybir.InstNoOp(
                                name=nc.get_next_instruction_name(),
                                engine=inst.engine,
                                sync_info=mybir.SyncInfo(on_wait=[w], on_update=[]),
                            )
                        )
                    inst.sync_info = mybir.SyncInfo(
                        on_wait=[waits[-1]], on_update=list(si.on_update)
                    )
                    changed = True
                new_insts.append(inst)
            if changed:
                bb.instructions = new_insts


def _build_program():
    key = ("prog_v2",)
    if key in _CACHE:
        return _CACHE[key]

    import concourse.bass as bass
    import concourse.mybir as mybir
    import concourse.tile as tile

    f32 = mybir.dt.float32
    bf16 = mybir.dt.bfloat16

    nc = bass.Bass()
    xt = nc.declare_dram_parameter("xt", [N_FEAT, RPC], bf16, isOutput=False)
    pgd = nc.declare_dram_parameter("pgd", [128, RPC], bf16, isOutput=False)
    wkb = nc.declare_dram_parameter("wkb", [128, 1024], bf16, isOutput=False)
    bkf = nc.declare_dram_parameter("bkf", [128, 4], f32, isOutput=False)
    cht = nc.declare_dram_parameter("cht", [2, 128, RPC], bf16, isOutput=True)

    AF = mybir.ActivationFunctionType

    with tile.TileContext(nc) as tc:
        with (
            tc.tile_pool(name="const", bufs=1) as cpool,
            tc.tile_pool(name="xin", bufs=2) as xpool,
            tc.tile_pool(name="gin", bufs=2) as gpool,
            tc.tile_pool(name="h1", bufs=6) as h1pool,
            tc.tile_pool(name="cout", bufs=2) as opool,
            tc.tile_pool(name="ps1", bufs=2, space="PSUM") as ps1,
            tc.tile_pool(name="ps2", bufs=2, space="PSUM") as ps2,
        ):
            wks = cpool.tile([128, 1024], bf16)
            nc.sync.dma_start(wks[:], wkb[:, :])
            bks = cpool.tile([128, 4], f32)
            nc.sync.dma_start(bks[:], bkf[:, :])
            w1xs = wks[:, 0:256]
            w2a = wks[:, 256:512]
            w2b = wks[:, 512:768]
            w1gs = wks[:, 768:896]
            rcs = wks[:, 896:1024]
            b1s = bks[:, 0:2]
            b2s = bks[:, 2:4]

            # PE warmup: ~3.5us of dummy matmuls on a never-written scratch
            # tile (values irrelevant, output discarded) so the HAM clock
            # gate reaches 8/8 before the first data tiles land. No DMA
            # dependency: starts right after the framework prologue.
            junk = cpool.tile([128, 2 * HID], bf16)
            nc.gpsimd.memset(junk[:, :], 0.0)
            warm = ps1.tile([128, 2 * GROUP], f32, tag="h1ps", name="warm")
            for _ in range(24):
                nc.tensor.matmul(
                    warm[:, 0:HID], junk[:, 0:128], junk[:, 256:512],
                    start=True, stop=True, skip_group_check=True,
                )

            # Software-pipelined emission (1-group skew): emit mm1(g)+lrelu(g)
            # BEFORE mm2(g-1). Scheduler priorities follow emission order and
            # the PE queue executes in order, so this keeps ready mm1 work in
            # front of mm2 instructions that wait on the lrelu chain.
            NG = N_SG * SGROUP
            sg_state = {}       # sg -> [og tile, og3 view, n_dve_done]
            prev = None
            prev2 = None

            def emit_resid(st):
                xs_, h2ps = st["xs"], st["h2ps"]
                # residual K=64 pair (depends only on xg); emitted right
                # after mm1's pg pair so the two pairs sit adjacent in the
                # PE stream and their weight-buffer stalls merge
                nc.tensor.matmul(
                    h2ps[0], rcs[0:64, :], xs_[0:64, :],
                    start=True, stop=False, tile_position=(0, 0),
                )
                nc.tensor.matmul(
                    h2ps[1], rcs[64:128, :], xs_[64:128, :],
                    start=True, stop=False, tile_position=(64, 0),
                )

            def emit_mm2(st):
                h1, h2ps, og3, cs = (
                    st["h1"], st["h2ps"], st["og3"], st["cs"]
                )
                for oh in range(2):
                    nc.tensor.matmul(
                        h2ps[oh], w2a[:, oh * 128:(oh + 1) * 128],
                        h1[0][:, :], start=False, stop=False,
                    )
                for oh in range(2):
                    nc.tensor.matmul(
                        h2ps[oh], w2b[:, oh * 128:(oh + 1) * 128],
                        h1[1][:, :], start=False, stop=True,
                    )
                # + b2 (per-partition here) while casting to bf16
                for oh in range(2):
                    nc.vector.tensor_scalar_add(
                        og3[:, oh, cs], h2ps[oh], b2s[:, oh:oh + 1],
                    )
                # close out the supergroup's output DMA after its last group
                sgp = st["sg"]
                sg_state[sgp][2] += 1
                ndone = sg_state[sgp][2]
                if sgp == N_SG - 1:
                    hw = GROUP
                    cp0 = sgp * SGROUP * GROUP + (ndone - 1) * hw
                    cp1 = cp0 + hw
                    nc.sync.dma_start(
                        cht[:, :, cp0:cp1].rearrange("oh p c -> p oh c"),
                        sg_state[sgp][1][:, :, (ndone - 1) * hw:ndone * hw],
                    )
                    if ndone == SGROUP:
                        del sg_state[sgp]
                elif ndone in (SGROUP // 2, SGROUP):
                    half = 0 if ndone == SGROUP // 2 else 1
                    hw = SGROUP * GROUP // 2
                    cp0 = sgp * SGROUP * GROUP + half * hw
                    cp1 = cp0 + hw
                    nc.sync.dma_start(
                        cht[:, :, cp0:cp1].rearrange("oh p c -> p oh c"),
                        sg_state[sgp][1][:, :, half * hw:(half + 1) * hw],
                    )
                    if ndone == SGROUP:
                        del sg_state[sgp]

            for g in range(NG):
                sg, gl = divmod(g, SGROUP)
                if gl == 0:
                    c0 = sg * SGROUP * GROUP
                    c1 = c0 + SGROUP * GROUP
                    gg = gpool.tile([128, SGROUP * GROUP], bf16, tag="gg")
                    nc.gpsimd.dma_start(gg[:, :], pgd[:, c0:c1])
                    xg = xpool.tile([128, SGROUP * GROUP], bf16, tag="xg")
                    nc.gpsimd.dma_start(xg[:, :], xt[:, c0:c1])
                    og = opool.tile([128, 2 * SGROUP * GROUP], bf16, tag="og")
                    og3 = og[:, :].rearrange("p (oh c) -> p oh c", oh=2)
                    sg_state[sg] = [og, og3, 0]

                cs = slice(gl * GROUP, (gl + 1) * GROUP)
                xs_ = xg[:, cs]
                gs_ = gg[:, cs]

                # ---- mm1: h1T[m] = W1x_m^T @ xT + W1g_m^T @ pgT ----
                # The K=64 pair starts each accumulation group: it only
                # depends on the input DMA, so the scheduler can issue the
                # two halves adjacently (concurrent row-tiles).
                h1ps2 = ps1.tile([128, 2 * GROUP], f32, tag="h1ps")
                h1ps = [h1ps2[:, m_ * GROUP:(m_ + 1) * GROUP]
                        for m_ in range(2)]
                nc.tensor.matmul(
                    h1ps[0], w1gs[0:64, :], gs_[0:64, :],
                    start=True, stop=False, tile_position=(0, 0),
                )
                nc.tensor.matmul(
                    h1ps[1], w1gs[64:128, :], gs_[64:128, :],
                    start=True, stop=False, tile_position=(64, 0),
                )
                if prev2 is not None:
                    emit_resid(prev2)
                for m in range(2):
                    nc.tensor.matmul(
                        h1ps[m], w1xs[:, m * 128:(m + 1) * 128], xs_,
                        start=False, stop=True,
                    )

                # ---- leaky-relu + b1 (per-partition bias), cast bf16 ----
                h1 = [h1pool.tile([128, GROUP], bf16, tag="h1sb",
                                  name=f"h1sb{m_}") for m_ in range(2)]
                for m in range(2):
                    nc.scalar.activation(
                        h1[m][:, :], h1ps[m], AF.Lrelu,
                        bias=b1s[:, m:m + 1], scale=1.0, alpha=0.01,
                    )

                # mm2 tiles for this group (allocated now, filled next iter)
                h2ps2 = ps2.tile([128, 2 * GROUP], f32, tag="h2ps")
                h2ps = [h2ps2[:, o_ * GROUP:(o_ + 1) * GROUP]
                        for o_ in range(2)]
                cur = {"sg": sg, "cs": cs, "xs": xs_, "h1": h1,
                       "h2ps": h2ps, "og3": og3}

                if prev2 is not None:
                    emit_mm2(prev2)
                prev2 = prev
                prev = cur

            emit_resid(prev2)
            emit_mm2(prev2)
            emit_resid(prev)
            emit_mm2(prev)

    import concourse.mybir as mybir
    _split_multiwait(nc, mybir)
    _CACHE[key] = nc
    return nc


def _host_prep(x, global_features, W1, b1, W2, b2, idxs_level, parents_idxs):
    bf = ml_dtypes.bfloat16
    x = np.ascontiguousarray(np.asarray(x, dtype=np.float32))
    G = np.asarray(global_features, dtype=np.float32)
    W1 = np.asarray(W1, dtype=np.float32)
    b1 = np.asarray(b1, dtype=np.float32)
    W2 = np.asarray(W2, dtype=np.float32)
    b2 = np.asarray(b2, dtype=np.float32)
    idxs = np.asarray(idxs_level)
    pidx = np.asarray(parents_idxs)

    if np.array_equal(idxs, np.arange(ROWS, dtype=idxs.dtype)):
        xg = x
    else:  # general gather fallback (host)
        xg = np.ascontiguousarray(x[idxs])

    # per-core transposed x: [8, 128, RPC]
    xt = np.ascontiguousarray(
        xg.reshape(N_CORES, RPC, N_FEAT).transpose(0, 2, 1)
    ).astype(bf)
    # per-core transposed gathered globals, duplicated into both halves
    pg = G[pidx % BATCH]                                  # [ROWS, 64]
    pgt = pg.reshape(N_CORES, RPC, N_GLOBAL).transpose(0, 2, 1).astype(bf)
    pgd = np.ascontiguousarray(np.concatenate([pgt, pgt], axis=1))  # [8,128,RPC]

    w1x = W1[0:128, :]                                     # [128, 256]
    w1g = np.concatenate([W1[128:192, 0:128], W1[128:192, 128:256]], axis=0)
    rp = np.zeros((64, 128), dtype=np.float32)
    rp[np.arange(128) // 2, np.arange(128)] = 1.0
    rcd = np.concatenate([rp, rp], axis=0)
    wkb = np.ascontiguousarray(np.concatenate(
        [w1x, W2[0:128, :], W2[128:256, :], w1g, rcd], axis=1
    )).astype(bf)                                          # [128, 1024]
    bkf = np.ascontiguousarray(np.concatenate(
        [b1.reshape(2, 128).T, b2.reshape(2, 128).T], axis=1
    ).astype(np.float32))                                  # [128, 4]

    in_maps = []
    for c in range(N_CORES):
        in_maps.append({
            "xt": xt[c],
            "pgd": pgd[c],
            "wkb": wkb,
            "bkf": bkf,
        })
    return x, in_maps


def kernel(x, global_features, W1, b1, W2, b2, idxs_level, parents_idxs,
           _trace=False, _trace_kwargs=None):
    from concourse.bass_utils import run_bass_kernel_spmd

    x_np, in_maps = _host_prep(
        x, global_features, W1, b1, W2, b2, idxs_level, parents_idxs
    )
    nc = _build_program()
    res = run_bass_kernel_spmd(
        nc, in_maps, list(range(N_CORES)),
        trace=_trace, **(_trace_kwargs or {}),
    )
    # cht[core][oh, f, pl*1024 + b] -> children[(core*32 + pl)*2 + oh, b, f]
    parts = []
    for c in range(N_CORES):
        a = np.asarray(res.results[c]["cht"])              # [2, 128, RPC] bf16
        a = a.reshape(2, 128, PPC, BATCH).transpose(2, 0, 3, 1)
        parts.append(a.reshape(CPC, N_FEAT).astype(np.float32))
    children = np.concatenate(parts, axis=0)
    out = np.concatenate([x_np, children], axis=0)
    if _trace:
        kernel.last_result = res
    return out
